# revision 15
# baseline (speedup 1.0000x reference)
"""Trainium2 Bass kernel for nn_ContrastLoss (supervised-contrastive loss).

Reference computation (B=1024, D=128, C=100, K=32768, N=B+K=33792):
    l   = concat(labels, queue_label.T)          # [N, C]
    w   = labels @ l.T                           # [B, N] shared-class counts
    sim = query @ concat(keys, queue.T).T / T    # [B, N]
    logits = sim - rowmax(sim)
    denom  = sum(exp(logits) * logits_mask, 1)   # logits_mask zeros keys-diag
    loss = -(T/BT) * sqrt(w/max(w)) * (logits - log(denom))

Restructurings:
  * Pure data-parallel over the B (row) dim: core c owns rows
    [c*128, (c+1)*128) and computes ALL N columns.  No collectives; each
    core's execution is fully independent of its peers.
  * max(w) == max_i rowsum(labels_i) exactly (binary labels, diag
    included) -> computed on host from the labels input and baked in.
  * Softmax stabilizer = 1.0 constant (inputs are L2-normalized so
    raw = q.d in [-1, 1]); kills the rowmax pass.
  * The self-diagonal removal from the denominator is a per-row dot
    product exp((q_i.k_i - m)/T), not a masked pass over the matrix.
  * Final algebra:  loss = (tc - raw) * sT  with
        tc = m + T*ln(denom)   (per-row scalar)
        sT = sqrt(w / wmax) / BT   (ACT Sqrt of the w-matmul PSUM)
    Phase A stores -raw as bf16 (DVE copy from PSUM) so the final op is
    ONE all-16-bit DVE scalar_tensor_tensor (2x mode):
        out = (-raw + tc) * sT.
  * bf16 sim matmul (4x faster than fp32 on the PE), fp8 w-matmul (0/1
    labels are exact in fp8), bf16 output (halves HBM write traffic).
"""

import numpy as np
import ml_dtypes

import concourse.bass as bass
import concourse.mybir as mybir
import concourse.tile as tile
from concourse import bacc
from concourse.bass_utils import run_bass_kernel_spmd

F32 = mybir.dt.float32
BF16 = mybir.dt.bfloat16
FP8 = mybir.dt.float8e4
ALU = mybir.AluOpType
ACTF = mybir.ActivationFunctionType

B, D, C, KQ = 1024, 128, 100, 32768
N = B + KQ                  # 33792 similarity columns
NCORES = 8
ROWS = B // NCORES          # 128 rows per core
STAB = 1.0                  # softmax stabilizer m (raw sim values in [-1, 1])

CH = 2048                   # main chunk: 4 matmuls of 512, 4 PSUM banks
CHUNKS = [(i * CH, CH) for i in range(N // CH)] + (
    [(N - N % CH, N % CH)] if N % CH else [])
# N = 33792 = 16*2048 + 1024


def _build_nc(Tf: float, BTf: float, wmax: float):
    nc = bacc.Bacc("TRN2", target_bir_lowering=False, debug=False,
                   num_devices=NCORES)

    qTb_d = nc.dram_tensor("qTb", [D, ROWS], BF16, kind="ExternalInput")
    labTb_d = nc.dram_tensor("labTb", [C, ROWS], FP8, kind="ExternalInput")
    qrow_d = nc.dram_tensor("qrow", [ROWS, D], BF16, kind="ExternalInput")
    krow_d = nc.dram_tensor("krow", [ROWS, D], BF16, kind="ExternalInput")
    rsim_d = nc.dram_tensor("rsim", [D, N], BF16, kind="ExternalInput")
    rw_d = nc.dram_tensor("rw", [C, N], FP8, kind="ExternalInput")
    out_d = nc.dram_tensor("out", [ROWS, N], BF16, kind="ExternalOutput")

    sq_scale = 1.0 / (BTf * BTf * max(wmax, 1.0))

    with tile.TileContext(nc) as tc:
        with (
            tc.tile_pool(name="const", bufs=1) as const,
            tc.tile_pool(name="rsw", bufs=3) as rsw,
            tc.tile_pool(name="escr", bufs=1) as escr_p,
            tc.tile_pool(name="sT", bufs=2) as sT_p,
            tc.tile_pool(name="outp", bufs=3) as outp,
            tc.tile_pool(name="psum", bufs=2, space="PSUM") as psum,
        ):
            # ---- small input loads (qTb + first sim chunks go first so the
            # first matmul can start as early as possible) ------------------
            qTb = const.tile([D, ROWS], BF16)
            nc.sync.dma_start(out=qTb[:], in_=qTb_d[:])
            rs_pre = []
            for k in range(2):
                base, n = CHUNKS[k]
                rs = rsw.tile([D, n], BF16, tag="rs", name=f"rs{k}")
                nc.gpsimd.dma_start(out=rs[:], in_=rsim_d[:, base:base + n])
                rs_pre.append(rs)
            labTb = const.tile([C, ROWS], FP8)
            nc.sync.dma_start(out=labTb[:], in_=labTb_d[:])
            qrow = const.tile([ROWS, D], BF16)
            nc.sync.dma_start(out=qrow[:], in_=qrow_d[:])
            krow = const.tile([ROWS, D], BF16)
            nc.sync.dma_start(out=krow[:], in_=krow_d[:])

            ebias = const.tile([ROWS, 1], F32)
            nc.vector.memset(ebias, -STAB / Tf)
            zbias = const.tile([ROWS, 1], F32)
            nc.vector.memset(zbias, 0.0)

            # ---- self-diagonal term: e_self = exp((q_i.k_i - m)/T) -------
            qkp = const.tile([ROWS, D], F32)
            nc.vector.tensor_mul(qkp[:], qrow[:], krow[:])
            qks = const.tile([ROWS, 1], F32)
            nc.vector.tensor_reduce(qks[:], qkp[:], axis=mybir.AxisListType.X,
                                    op=ALU.add)
            eself = const.tile([ROWS, 1], F32)
            nc.scalar.activation(eself[:], qks[:], ACTF.Exp,
                                 bias=ebias[:], scale=1.0 / Tf)

            # ---- phase A: sim matmul -> rowsum(exp); store -raw bf16 -----
            rawn = const.tile([ROWS, N], BF16)
            acc = const.tile([ROWS, len(CHUNKS)], F32)
            for k, (base, n) in enumerate(CHUNKS):
                if k < 2:
                    rs = rs_pre[k]
                else:
                    rs = rsw.tile([D, n], BF16, tag="rs")
                    nc.gpsimd.dma_start(out=rs[:], in_=rsim_d[:, base:base + n])
                ps = psum.tile([ROWS, n], F32, tag="pa")
                for o in range(0, n, 512):
                    nc.tensor.matmul(ps[:, o:o + 512], qTb[:],
                                     rs[:, o:o + 512],
                                     start=True, stop=True)
                # Single PSUM reader (Tile serializes same-tile readers, so a
                # second reader would sit on the PSUM-recycle path): evacuate
                # +raw to SBUF (plain copy), then Exp reads the bf16 copy
                # asynchronously.  The final combine emits -loss and the host
                # negates during reassembly.
                nc.vector.tensor_copy(out=rawn[:, base:base + n], in_=ps[:])
                e_scr = escr_p.tile([ROWS, n], BF16, tag="e")
                nc.scalar.activation(e_scr[:], rawn[:, base:base + n], ACTF.Exp,
                                     bias=ebias[:], scale=1.0 / Tf,
                                     accum_out=acc[:, k:k + 1])

            # ---- denominator and per-row constant tc = m + T*ln(denom) ---
            dnsum = const.tile([ROWS, 1], F32)
            nc.vector.tensor_reduce(dnsum[:], acc[:], axis=mybir.AxisListType.X,
                                    op=ALU.add)
            denom = const.tile([ROWS, 1], F32)
            nc.vector.tensor_sub(denom[:], dnsum[:], eself[:])
            lnd = const.tile([ROWS, 1], F32)
            nc.scalar.activation(lnd[:], denom[:], ACTF.Ln, bias=zbias[:])
            tc_row = const.tile([ROWS, 1], F32)
            nc.vector.tensor_scalar(tc_row[:], lnd[:], Tf, STAB,
                                    op0=ALU.mult, op1=ALU.add)
            # Sqrt takes its scale from an AP derived from lnd purely to pin
            # the ACT queue order Ln -> Sqrt(0..): otherwise the scheduler
            # runs an early Sqrt before Ln and thrashes the ACT table set.
            sq_ap = const.tile([ROWS, 1], F32)
            nc.vector.tensor_scalar(sq_ap[:], lnd[:], 0.0, sq_scale,
                                    op0=ALU.mult, op1=ALU.add)

            # ---- phase B: w matmul -> sT; out = (-raw + tc) * sT ---------
            for k, (base, n) in enumerate(CHUNKS):
                rwt = rsw.tile([C, n], FP8, tag="rw")
                nc.sync.dma_start(out=rwt[:], in_=rw_d[:, base:base + n])
                psw = psum.tile([ROWS, n], F32, tag="pa")
                for o in range(0, n, 512):
                    nc.tensor.matmul(psw[:, o:o + 512], labTb[:],
                                     rwt[:, o:o + 512],
                                     start=True, stop=True)
                sT = sT_p.tile([ROWS, n], BF16, tag="s")
                nc.scalar.activation(sT[:], psw[:], ACTF.Sqrt,
                                     bias=zbias[:], scale=sq_ap[:])
                o_t = outp.tile([ROWS, n], BF16, tag="o")
                # o = (raw - tc) * sT = -loss; negated on the host.
                nc.vector.scalar_tensor_tensor(
                    o_t[:], rawn[:, base:base + n], tc_row[:], sT[:],
                    op0=ALU.subtract, op1=ALU.mult,
                )
                # Output DMAs issue from the idle Pool sequencer so they don't
                # queue behind the input DMAs on SP.
                nc.gpsimd.dma_start(out=out_d[:, base:base + n], in_=o_t[:])
    nc.compile()
    return nc


def _host_prep(query, keys, labels, queue, queue_label):
    bf16 = ml_dtypes.bfloat16
    fp8 = ml_dtypes.float8_e4m3
    query = np.asarray(query, np.float32)
    keys = np.asarray(keys, np.float32)
    labels = np.asarray(labels, np.float32)
    queue = np.asarray(queue, np.float32)
    queue_label = np.asarray(queue_label, np.float32)

    qT = query.T                                        # [D, B]
    labT = labels.T                                     # [C, B]
    rsim = np.ascontiguousarray(
        np.concatenate([keys.T, queue], axis=1)).astype(bf16)   # [D, N]
    rw = np.ascontiguousarray(
        np.concatenate([labT, queue_label], axis=1)).astype(fp8)  # [C, N]

    in_maps = []
    for c in range(NCORES):
        blk = slice(c * ROWS, (c + 1) * ROWS)
        in_maps.append({
            "qTb": np.ascontiguousarray(qT[:, blk]).astype(bf16),
            "labTb": np.ascontiguousarray(labT[:, blk]).astype(fp8),
            "qrow": np.ascontiguousarray(query[blk]).astype(bf16),
            "krow": np.ascontiguousarray(keys[blk]).astype(bf16),
            "rsim": rsim,
            "rw": rw,
        })
    return in_maps


def _gather_output(results):
    out = np.empty((B, N), np.float32)
    for c in range(NCORES):
        out[c * ROWS:(c + 1) * ROWS, :] = -results[c]["out"].astype(np.float32)
    return out


def kernel(query, keys, labels, queue, queue_label, K, T, BT, **_unused):
    Tf = float(np.asarray(T))
    BTf = float(np.asarray(BT))
    labels = np.asarray(labels, np.float32)
    wmax = float(labels.sum(axis=1).max())
    nc = _build_nc(Tf, BTf, wmax)
    in_maps = _host_prep(query, keys, labels, queue, queue_label)
    res = run_bass_kernel_spmd(nc, in_maps, list(range(NCORES)))
    return _gather_output(res.results)


# Re-usable entry for test.py: returns (output, BassKernelResults) so the
# harness there can pull exec_time_ns / profile out of a traced run.
def kernel_traced(query, keys, labels, queue, queue_label, K, T, BT,
                  trace=False, **run_kwargs):
    Tf = float(np.asarray(T))
    BTf = float(np.asarray(BT))
    labels = np.asarray(labels, np.float32)
    wmax = float(labels.sum(axis=1).max())
    nc = _build_nc(Tf, BTf, wmax)
    in_maps = _host_prep(query, keys, labels, queue, queue_label)
    res = run_bass_kernel_spmd(nc, in_maps, list(range(NCORES)),
                               trace=trace, **run_kwargs)
    return _gather_output(res.results), res


# revision 16
# speedup vs baseline: 1.0040x; 1.0040x over previous
"""Trainium2 Bass kernel for nn_ContrastLoss (supervised-contrastive loss).

Reference computation (B=1024, D=128, C=100, K=32768, N=B+K=33792):
    l   = concat(labels, queue_label.T)          # [N, C]
    w   = labels @ l.T                           # [B, N] shared-class counts
    sim = query @ concat(keys, queue.T).T / T    # [B, N]
    logits = sim - rowmax(sim)
    denom  = sum(exp(logits) * logits_mask, 1)   # logits_mask zeros keys-diag
    loss = -(T/BT) * sqrt(w/max(w)) * (logits - log(denom))

Restructurings:
  * Pure data-parallel over the B (row) dim: core c owns rows
    [c*128, (c+1)*128) and computes ALL N columns.  No collectives; each
    core's execution is fully independent of its peers.
  * max(w) == max_i rowsum(labels_i) exactly (binary labels, diag
    included) -> computed on host from the labels input and baked in.
  * Softmax stabilizer = 1.0 constant (inputs are L2-normalized so
    raw = q.d in [-1, 1]); kills the rowmax pass.
  * The self-diagonal removal from the denominator is a per-row dot
    product exp((q_i.k_i - m)/T), not a masked pass over the matrix.
  * Final algebra:  loss = (tc - raw) * sT  with
        tc = m + T*ln(denom)   (per-row scalar)
        sT = sqrt(w / wmax) / BT   (ACT Sqrt of the w-matmul PSUM)
    Phase A stores -raw as bf16 (DVE copy from PSUM) so the final op is
    ONE all-16-bit DVE scalar_tensor_tensor (2x mode):
        out = (-raw + tc) * sT.
  * bf16 sim matmul (4x faster than fp32 on the PE), fp8 w-matmul (0/1
    labels are exact in fp8), bf16 output (halves HBM write traffic).
"""

import numpy as np
import ml_dtypes

import concourse.bass as bass
import concourse.mybir as mybir
import concourse.tile as tile
from concourse import bacc
from concourse.bass_utils import run_bass_kernel_spmd

F32 = mybir.dt.float32
BF16 = mybir.dt.bfloat16
FP8 = mybir.dt.float8e4
ALU = mybir.AluOpType
ACTF = mybir.ActivationFunctionType

B, D, C, KQ = 1024, 128, 100, 32768
N = B + KQ                  # 33792 similarity columns
NCORES = 8
ROWS = B // NCORES          # 128 rows per core
STAB = 1.0                  # softmax stabilizer m (raw sim values in [-1, 1])

CH = 2048                   # main chunk: 4 matmuls of 512, 4 PSUM banks
CHUNKS = [(i * CH, CH) for i in range(N // CH)] + (
    [(N - N % CH, N % CH)] if N % CH else [])
# N = 33792 = 16*2048 + 1024


def _build_nc(Tf: float, BTf: float, wmax: float):
    nc = bacc.Bacc("TRN2", target_bir_lowering=False, debug=False,
                   num_devices=NCORES)

    qTb_d = nc.dram_tensor("qTb", [D, ROWS], BF16, kind="ExternalInput")
    labTb_d = nc.dram_tensor("labTb", [C, ROWS], FP8, kind="ExternalInput")
    qrow_d = nc.dram_tensor("qrow", [ROWS, D], BF16, kind="ExternalInput")
    krow_d = nc.dram_tensor("krow", [ROWS, D], BF16, kind="ExternalInput")
    rsim_d = nc.dram_tensor("rsim", [D, N], BF16, kind="ExternalInput")
    rw_d = nc.dram_tensor("rw", [C, N], FP8, kind="ExternalInput")
    out_d = nc.dram_tensor("out", [ROWS, N], BF16, kind="ExternalOutput")

    sq_scale = 1.0 / (BTf * BTf * max(wmax, 1.0))

    with tile.TileContext(nc) as tc:
        with (
            tc.tile_pool(name="const", bufs=1) as const,
            tc.tile_pool(name="rsw", bufs=3) as rsw,
            tc.tile_pool(name="escr", bufs=1) as escr_p,
            tc.tile_pool(name="sT", bufs=2) as sT_p,
            tc.tile_pool(name="outp", bufs=3) as outp,
            tc.tile_pool(name="psum", bufs=2, space="PSUM") as psum,
        ):
            # ---- small input loads (qTb + first sim chunks go first so the
            # first matmul can start as early as possible) ------------------
            qTb = const.tile([D, ROWS], BF16)
            nc.sync.dma_start(out=qTb[:], in_=qTb_d[:])
            rs_pre = []
            for k in range(2):
                base, n = CHUNKS[k]
                rs = rsw.tile([D, n], BF16, tag="rs", name=f"rs{k}")
                nc.sync.dma_start(out=rs[:], in_=rsim_d[:, base:base + n])
                rs_pre.append(rs)
            labTb = const.tile([C, ROWS], FP8)
            nc.sync.dma_start(out=labTb[:], in_=labTb_d[:])
            qrow = const.tile([ROWS, D], BF16)
            nc.sync.dma_start(out=qrow[:], in_=qrow_d[:])
            krow = const.tile([ROWS, D], BF16)
            nc.sync.dma_start(out=krow[:], in_=krow_d[:])

            ebias = const.tile([ROWS, 1], F32)
            nc.vector.memset(ebias, -STAB / Tf)
            zbias = const.tile([ROWS, 1], F32)
            nc.vector.memset(zbias, 0.0)

            # ---- self-diagonal term: e_self = exp((q_i.k_i - m)/T) -------
            qkp = const.tile([ROWS, D], F32)
            nc.vector.tensor_mul(qkp[:], qrow[:], krow[:])
            qks = const.tile([ROWS, 1], F32)
            nc.vector.tensor_reduce(qks[:], qkp[:], axis=mybir.AxisListType.X,
                                    op=ALU.add)
            eself = const.tile([ROWS, 1], F32)
            nc.scalar.activation(eself[:], qks[:], ACTF.Exp,
                                 bias=ebias[:], scale=1.0 / Tf)

            # ---- phase A: sim matmul -> rowsum(exp); store -raw bf16 -----
            rawn = const.tile([ROWS, N], BF16)
            acc = const.tile([ROWS, len(CHUNKS)], F32)
            for k, (base, n) in enumerate(CHUNKS):
                if k < 2:
                    rs = rs_pre[k]
                else:
                    rs = rsw.tile([D, n], BF16, tag="rs")
                    nc.sync.dma_start(out=rs[:], in_=rsim_d[:, base:base + n])
                ps = psum.tile([ROWS, n], F32, tag="pa")
                for o in range(0, n, 512):
                    nc.tensor.matmul(ps[:, o:o + 512], qTb[:],
                                     rs[:, o:o + 512],
                                     start=True, stop=True)
                # Single PSUM reader (Tile serializes same-tile readers, so a
                # second reader would sit on the PSUM-recycle path): evacuate
                # +raw to SBUF (plain copy), then Exp reads the bf16 copy
                # asynchronously.  The final combine emits -loss and the host
                # negates during reassembly.
                nc.vector.tensor_copy(out=rawn[:, base:base + n], in_=ps[:])
                e_scr = escr_p.tile([ROWS, n], BF16, tag="e")
                nc.scalar.activation(e_scr[:], rawn[:, base:base + n], ACTF.Exp,
                                     bias=ebias[:], scale=1.0 / Tf,
                                     accum_out=acc[:, k:k + 1])

            # ---- denominator and per-row constant tc = m + T*ln(denom) ---
            dnsum = const.tile([ROWS, 1], F32)
            nc.vector.tensor_reduce(dnsum[:], acc[:], axis=mybir.AxisListType.X,
                                    op=ALU.add)
            denom = const.tile([ROWS, 1], F32)
            nc.vector.tensor_sub(denom[:], dnsum[:], eself[:])
            lnd = const.tile([ROWS, 1], F32)
            nc.scalar.activation(lnd[:], denom[:], ACTF.Ln, bias=zbias[:])
            tc_row = const.tile([ROWS, 1], F32)
            nc.vector.tensor_scalar(tc_row[:], lnd[:], Tf, STAB,
                                    op0=ALU.mult, op1=ALU.add)
            # Sqrt takes its scale from an AP derived from lnd purely to pin
            # the ACT queue order Ln -> Sqrt(0..): otherwise the scheduler
            # runs an early Sqrt before Ln and thrashes the ACT table set.
            sq_ap = const.tile([ROWS, 1], F32)
            nc.vector.tensor_scalar(sq_ap[:], lnd[:], 0.0, sq_scale,
                                    op0=ALU.mult, op1=ALU.add)

            # ---- phase B: w matmul -> sT; out = (-raw + tc) * sT ---------
            for k, (base, n) in enumerate(CHUNKS):
                rwt = rsw.tile([C, n], FP8, tag="rw")
                nc.sync.dma_start(out=rwt[:], in_=rw_d[:, base:base + n])
                psw = psum.tile([ROWS, n], F32, tag="pa")
                for o in range(0, n, 512):
                    nc.tensor.matmul(psw[:, o:o + 512], labTb[:],
                                     rwt[:, o:o + 512],
                                     start=True, stop=True)
                sT = sT_p.tile([ROWS, n], BF16, tag="s")
                nc.scalar.activation(sT[:], psw[:], ACTF.Sqrt,
                                     bias=zbias[:], scale=sq_ap[:])
                o_t = outp.tile([ROWS, n], BF16, tag="o")
                # o = (raw - tc) * sT = -loss; negated on the host.
                nc.vector.scalar_tensor_tensor(
                    o_t[:], rawn[:, base:base + n], tc_row[:], sT[:],
                    op0=ALU.subtract, op1=ALU.mult,
                )
                # Output DMAs issue from the idle Pool sequencer so they don't
                # queue behind the input DMAs on SP.
                nc.gpsimd.dma_start(out=out_d[:, base:base + n], in_=o_t[:])
    nc.compile()
    return nc


def _host_prep(query, keys, labels, queue, queue_label):
    bf16 = ml_dtypes.bfloat16
    fp8 = ml_dtypes.float8_e4m3
    query = np.asarray(query, np.float32)
    keys = np.asarray(keys, np.float32)
    labels = np.asarray(labels, np.float32)
    queue = np.asarray(queue, np.float32)
    queue_label = np.asarray(queue_label, np.float32)

    qT = query.T                                        # [D, B]
    labT = labels.T                                     # [C, B]
    rsim = np.ascontiguousarray(
        np.concatenate([keys.T, queue], axis=1)).astype(bf16)   # [D, N]
    rw = np.ascontiguousarray(
        np.concatenate([labT, queue_label], axis=1)).astype(fp8)  # [C, N]

    in_maps = []
    for c in range(NCORES):
        blk = slice(c * ROWS, (c + 1) * ROWS)
        in_maps.append({
            "qTb": np.ascontiguousarray(qT[:, blk]).astype(bf16),
            "labTb": np.ascontiguousarray(labT[:, blk]).astype(fp8),
            "qrow": np.ascontiguousarray(query[blk]).astype(bf16),
            "krow": np.ascontiguousarray(keys[blk]).astype(bf16),
            "rsim": rsim,
            "rw": rw,
        })
    return in_maps


def _gather_output(results):
    out = np.empty((B, N), np.float32)
    for c in range(NCORES):
        out[c * ROWS:(c + 1) * ROWS, :] = -results[c]["out"].astype(np.float32)
    return out


def kernel(query, keys, labels, queue, queue_label, K, T, BT, **_unused):
    Tf = float(np.asarray(T))
    BTf = float(np.asarray(BT))
    labels = np.asarray(labels, np.float32)
    wmax = float(labels.sum(axis=1).max())
    nc = _build_nc(Tf, BTf, wmax)
    in_maps = _host_prep(query, keys, labels, queue, queue_label)
    res = run_bass_kernel_spmd(nc, in_maps, list(range(NCORES)))
    return _gather_output(res.results)


# Re-usable entry for test.py: returns (output, BassKernelResults) so the
# harness there can pull exec_time_ns / profile out of a traced run.
def kernel_traced(query, keys, labels, queue, queue_label, K, T, BT,
                  trace=False, **run_kwargs):
    Tf = float(np.asarray(T))
    BTf = float(np.asarray(BT))
    labels = np.asarray(labels, np.float32)
    wmax = float(labels.sum(axis=1).max())
    nc = _build_nc(Tf, BTf, wmax)
    in_maps = _host_prep(query, keys, labels, queue, queue_label)
    res = run_bass_kernel_spmd(nc, in_maps, list(range(NCORES)),
                               trace=trace, **run_kwargs)
    return _gather_output(res.results), res


# revision 17
# speedup vs baseline: 1.0269x; 1.0229x over previous
"""Trainium2 Bass kernel for nn_ContrastLoss (supervised-contrastive loss).

Reference computation (B=1024, D=128, C=100, K=32768, N=B+K=33792):
    l   = concat(labels, queue_label.T)          # [N, C]
    w   = labels @ l.T                           # [B, N] shared-class counts
    sim = query @ concat(keys, queue.T).T / T    # [B, N]
    logits = sim - rowmax(sim)
    denom  = sum(exp(logits) * logits_mask, 1)   # logits_mask zeros keys-diag
    loss = -(T/BT) * sqrt(w/max(w)) * (logits - log(denom))

Restructurings:
  * Pure data-parallel over the B (row) dim: core c owns rows
    [c*128, (c+1)*128) and computes ALL N columns.  No collectives; each
    core's execution is fully independent of its peers.
  * max(w) == max_i rowsum(labels_i) exactly (binary labels, diag
    included) -> computed on host from the labels input and baked in.
  * Softmax stabilizer = 1.0 constant (inputs are L2-normalized so
    raw = q.d in [-1, 1]); kills the rowmax pass.
  * The self-diagonal removal from the denominator is a per-row dot
    product exp((q_i.k_i - m)/T), not a masked pass over the matrix.
  * Final algebra:  loss = (tc - raw) * sT  with
        tc = m + T*ln(denom)   (per-row scalar)
        sT = sqrt(w / wmax) / BT   (ACT Sqrt of the w-matmul PSUM)
    Phase A evacuates +raw to bf16 SBUF with a DVE tensor_copy — the
    SOLE reader of each PSUM chunk, so the PSUM recycle chain is just
    matmul -> copy (Tile serializes same-tile readers; a second reader
    would stall the pipeline).  Exp then reads the bf16 copy off-chain.
    Phase B emits  o = (raw - tc) * sT = -loss  in one DVE
    scalar_tensor_tensor; the host negates while reassembling.
  * bf16 sim matmul (4x faster than fp32 on the PE), fp8 w-matmul (0/1
    labels are exact in fp8), bf16 output (halves HBM write traffic).
  * Sqrt's scale comes from an AP derived from ln(denom) purely to pin
    the ACT queue order Ln -> Sqrt(0..) (avoids ACT-table thrash), and
    output DMAs issue from the idle Pool sequencer so they never queue
    behind input DMAs on SP.
"""

import numpy as np
import ml_dtypes

import concourse.bass as bass
import concourse.mybir as mybir
import concourse.tile as tile
from concourse import bacc
from concourse.bass_utils import run_bass_kernel_spmd

F32 = mybir.dt.float32
BF16 = mybir.dt.bfloat16
FP8 = mybir.dt.float8e4
ALU = mybir.AluOpType
ACTF = mybir.ActivationFunctionType

B, D, C, KQ = 1024, 128, 100, 32768
N = B + KQ                  # 33792 similarity columns
NCORES = 8
ROWS = B // NCORES          # 128 rows per core
STAB = 1.0                  # softmax stabilizer m (raw sim values in [-1, 1])

CH = 2048                   # main chunk: 4 matmuls of 512, 4 PSUM banks
CHUNKS = [(i * CH, CH) for i in range(N // CH)] + (
    [(N - N % CH, N % CH)] if N % CH else [])
# N = 33792 = 16*2048 + 1024


def _build_nc(Tf: float, BTf: float, wmax: float):
    nc = bacc.Bacc("TRN2", target_bir_lowering=False, debug=False,
                   num_devices=NCORES)

    qTb_d = nc.dram_tensor("qTb", [D, ROWS], BF16, kind="ExternalInput")
    labTb_d = nc.dram_tensor("labTb", [C, ROWS], FP8, kind="ExternalInput")
    qrow_d = nc.dram_tensor("qrow", [ROWS, D], BF16, kind="ExternalInput")
    krow_d = nc.dram_tensor("krow", [ROWS, D], BF16, kind="ExternalInput")
    rsim_d = nc.dram_tensor("rsim", [D, N], BF16, kind="ExternalInput")
    rw_d = nc.dram_tensor("rw", [C, N], FP8, kind="ExternalInput")
    out_d = nc.dram_tensor("out", [ROWS, N], BF16, kind="ExternalOutput")

    sq_scale = 1.0 / (BTf * BTf * max(wmax, 1.0))

    with tile.TileContext(nc) as tc:
        with (
            tc.tile_pool(name="const", bufs=1) as const,
            tc.tile_pool(name="rsw", bufs=3) as rsw,
            tc.tile_pool(name="escr", bufs=1) as escr_p,
            tc.tile_pool(name="sT", bufs=2) as sT_p,
            tc.tile_pool(name="outp", bufs=3) as outp,
            tc.tile_pool(name="psum", bufs=2, space="PSUM") as psum,
        ):
            # ---- small input loads (qTb + first sim chunks go first so the
            # first matmul can start as early as possible) ------------------
            qTb = const.tile([D, ROWS], BF16)
            nc.sync.dma_start(out=qTb[:], in_=qTb_d[:])
            rs_pre = []
            for k in range(2):
                base, n = CHUNKS[k]
                rs = rsw.tile([D, n], BF16, tag="rs", name=f"rs{k}")
                nc.sync.dma_start(out=rs[:], in_=rsim_d[:, base:base + n])
                rs_pre.append(rs)
            labTb = const.tile([C, ROWS], FP8)
            nc.sync.dma_start(out=labTb[:], in_=labTb_d[:])
            qrow = const.tile([ROWS, D], BF16)
            nc.sync.dma_start(out=qrow[:], in_=qrow_d[:])
            krow = const.tile([ROWS, D], BF16)
            nc.sync.dma_start(out=krow[:], in_=krow_d[:])

            ebias = const.tile([ROWS, 1], F32)
            nc.vector.memset(ebias, -STAB / Tf)
            zbias = const.tile([ROWS, 1], F32)
            nc.vector.memset(zbias, 0.0)

            # ---- self-diagonal term: e_self = exp((q_i.k_i - m)/T) -------
            qkp = const.tile([ROWS, D], F32)
            nc.vector.tensor_mul(qkp[:], qrow[:], krow[:])
            qks = const.tile([ROWS, 1], F32)
            nc.vector.tensor_reduce(qks[:], qkp[:], axis=mybir.AxisListType.X,
                                    op=ALU.add)
            eself = const.tile([ROWS, 1], F32)
            nc.scalar.activation(eself[:], qks[:], ACTF.Exp,
                                 bias=ebias[:], scale=1.0 / Tf)

            # ---- phase A: sim matmul -> rowsum(exp); store -raw bf16 -----
            rawn = const.tile([ROWS, N], BF16)
            acc = const.tile([ROWS, len(CHUNKS)], F32)
            for k, (base, n) in enumerate(CHUNKS):
                if k < 2:
                    rs = rs_pre[k]
                else:
                    rs = rsw.tile([D, n], BF16, tag="rs")
                    nc.sync.dma_start(out=rs[:], in_=rsim_d[:, base:base + n])
                ps = psum.tile([ROWS, n], F32, tag="pa")
                for o in range(0, n, 512):
                    nc.tensor.matmul(ps[:, o:o + 512], qTb[:],
                                     rs[:, o:o + 512],
                                     start=True, stop=True)
                # Single PSUM reader (Tile serializes same-tile readers, so a
                # second reader would sit on the PSUM-recycle path): evacuate
                # +raw to SBUF (plain copy), then Exp reads the bf16 copy
                # asynchronously.  The final combine emits -loss and the host
                # negates during reassembly.
                nc.vector.tensor_copy(out=rawn[:, base:base + n], in_=ps[:])
                e_scr = escr_p.tile([ROWS, n], BF16, tag="e")
                nc.scalar.activation(e_scr[:], rawn[:, base:base + n], ACTF.Exp,
                                     bias=ebias[:], scale=1.0 / Tf,
                                     accum_out=acc[:, k:k + 1])

            # ---- denominator and per-row constant tc = m + T*ln(denom) ---
            dnsum = const.tile([ROWS, 1], F32)
            nc.vector.tensor_reduce(dnsum[:], acc[:], axis=mybir.AxisListType.X,
                                    op=ALU.add)
            denom = const.tile([ROWS, 1], F32)
            nc.vector.tensor_sub(denom[:], dnsum[:], eself[:])
            lnd = const.tile([ROWS, 1], F32)
            nc.scalar.activation(lnd[:], denom[:], ACTF.Ln, bias=zbias[:])
            tc_row = const.tile([ROWS, 1], F32)
            nc.vector.tensor_scalar(tc_row[:], lnd[:], Tf, STAB,
                                    op0=ALU.mult, op1=ALU.add)
            # Sqrt takes its scale from an AP derived from lnd purely to pin
            # the ACT queue order Ln -> Sqrt(0..): otherwise the scheduler
            # runs an early Sqrt before Ln and thrashes the ACT table set.
            sq_ap = const.tile([ROWS, 1], F32)
            nc.vector.tensor_scalar(sq_ap[:], lnd[:], 0.0, sq_scale,
                                    op0=ALU.mult, op1=ALU.add)

            # ---- phase B: w matmul -> sT; out = (-raw + tc) * sT ---------
            for k, (base, n) in enumerate(CHUNKS):
                rwt = rsw.tile([C, n], FP8, tag="rw")
                nc.sync.dma_start(out=rwt[:], in_=rw_d[:, base:base + n])
                psw = psum.tile([ROWS, n], F32, tag="pa")
                for o in range(0, n, 512):
                    nc.tensor.matmul(psw[:, o:o + 512], labTb[:],
                                     rwt[:, o:o + 512],
                                     start=True, stop=True)
                sT = sT_p.tile([ROWS, n], BF16, tag="s")
                nc.scalar.activation(sT[:], psw[:], ACTF.Sqrt,
                                     bias=zbias[:], scale=sq_ap[:])
                o_t = outp.tile([ROWS, n], BF16, tag="o")
                # o = (raw - tc) * sT = -loss; negated on the host.
                nc.vector.scalar_tensor_tensor(
                    o_t[:], rawn[:, base:base + n], tc_row[:], sT[:],
                    op0=ALU.subtract, op1=ALU.mult,
                )
                # Output DMAs issue from the idle Pool sequencer so they don't
                # queue behind the input DMAs on SP.
                nc.gpsimd.dma_start(out=out_d[:, base:base + n], in_=o_t[:])
    nc.compile()
    return nc


def _host_prep(query, keys, labels, queue, queue_label):
    bf16 = ml_dtypes.bfloat16
    fp8 = ml_dtypes.float8_e4m3
    query = np.asarray(query, np.float32)
    keys = np.asarray(keys, np.float32)
    labels = np.asarray(labels, np.float32)
    queue = np.asarray(queue, np.float32)
    queue_label = np.asarray(queue_label, np.float32)

    qT = query.T                                        # [D, B]
    labT = labels.T                                     # [C, B]
    rsim = np.ascontiguousarray(
        np.concatenate([keys.T, queue], axis=1)).astype(bf16)   # [D, N]
    rw = np.ascontiguousarray(
        np.concatenate([labT, queue_label], axis=1)).astype(fp8)  # [C, N]

    in_maps = []
    for c in range(NCORES):
        blk = slice(c * ROWS, (c + 1) * ROWS)
        in_maps.append({
            "qTb": np.ascontiguousarray(qT[:, blk]).astype(bf16),
            "labTb": np.ascontiguousarray(labT[:, blk]).astype(fp8),
            "qrow": np.ascontiguousarray(query[blk]).astype(bf16),
            "krow": np.ascontiguousarray(keys[blk]).astype(bf16),
            "rsim": rsim,
            "rw": rw,
        })
    return in_maps


def _gather_output(results):
    out = np.empty((B, N), np.float32)
    for c in range(NCORES):
        out[c * ROWS:(c + 1) * ROWS, :] = -results[c]["out"].astype(np.float32)
    return out


def kernel(query, keys, labels, queue, queue_label, K, T, BT, **_unused):
    Tf = float(np.asarray(T))
    BTf = float(np.asarray(BT))
    labels = np.asarray(labels, np.float32)
    wmax = float(labels.sum(axis=1).max())
    nc = _build_nc(Tf, BTf, wmax)
    in_maps = _host_prep(query, keys, labels, queue, queue_label)
    res = run_bass_kernel_spmd(nc, in_maps, list(range(NCORES)))
    return _gather_output(res.results)


# Re-usable entry for test.py: returns (output, BassKernelResults) so the
# harness there can pull exec_time_ns / profile out of a traced run.
def kernel_traced(query, keys, labels, queue, queue_label, K, T, BT,
                  trace=False, **run_kwargs):
    Tf = float(np.asarray(T))
    BTf = float(np.asarray(BT))
    labels = np.asarray(labels, np.float32)
    wmax = float(labels.sum(axis=1).max())
    nc = _build_nc(Tf, BTf, wmax)
    in_maps = _host_prep(query, keys, labels, queue, queue_label)
    res = run_bass_kernel_spmd(nc, in_maps, list(range(NCORES)),
                               trace=trace, **run_kwargs)
    return _gather_output(res.results), res
